# revision 22
# baseline (speedup 1.0000x reference)
"""Trainium2 Bass kernel for nn_CrossAttention_9869834846452.

Cross-attention: learned query (+pos embed) attends over x-derived K/V.
  B=4, N=4096, C=768, H=12 heads, P=1024 queries, head_dim=64.

Sharding over 8 cores: core c = (b, hh) with b = c//2 (batch), hh = c%2
(head-group of 6 heads = 384 channels). Each core computes its 6 heads of
attention for its batch plus the partial output projection over its 384
channels; the host sums the two head-group partials per batch and adds the
projection bias.

Device-side layout choices (all transposes done on host during sharding):
  - x is passed transposed (xT [C, N]) so it serves both as the rhs of the
    kT-production matmul and as the lhsT of the v-production matmul.
  - K is produced directly in transposed form kT [384, N]; scores are
    computed transposed (sT [n, p] = kT_h.T-slices @ qT_h) so that the
    attention*V matmul needs no transposes anywhere.
  - The softmax denominator comes from a ones-column appended to V
    (v_aug [n, 65]); the AV matmul then yields [num; Z] in one pass.
  - Heads are processed in pairs occupying SBUF partitions 0-63 / 64-127,
    which row-packs the K=64 QK matmuls onto disjoint PE row-groups
    (they run concurrently in the array).
"""

import ml_dtypes
import numpy as np

import concourse.bass as bass
import concourse.tile as tile
from concourse import bacc, mybir
from concourse.bass_utils import run_bass_kernel_spmd

F32 = mybir.dt.float32
BF16 = mybir.dt.bfloat16
EXP = mybir.ActivationFunctionType.Exp

B, N, C, H, P, HD = 4, 4096, 768, 12, 1024, 64
SCALE = HD ** -0.5
HG = 384          # channels per head-group (6 heads)
NT = N // 128     # 32 n-tiles
NCH = N // 512    # 8 n-chunks
CCH = C // 128    # 6 contraction chunks

_PROGRAM = None
LAST_RESULTS = None


def _build_body(tc, debug=False):
    nc = tc.nc
    xT = nc.dram_tensor("xT", [C, N], BF16, kind="ExternalInput").ap()
    wkT = nc.dram_tensor("wkT", [C, HG], BF16, kind="ExternalInput").ap()
    wvT = nc.dram_tensor("wvT", [C, 390], BF16, kind="ExternalInput").ap()
    qT = nc.dram_tensor("qT", [HG, P], BF16, kind="ExternalInput").ap()
    projT = nc.dram_tensor("projT", [6, 64, C], BF16, kind="ExternalInput").ap()
    out = nc.dram_tensor("out", [P, C], F32, kind="ExternalOutput").ap()
    if debug:
        kT_dump = nc.dram_tensor("kT_dump", [128, 3 * N], BF16, kind="ExternalOutput").ap()
        v_dump = nc.dram_tensor("v_dump", [128, NT * 390], BF16, kind="ExternalOutput").ap()
        at_dump = nc.dram_tensor("at_dump", [128, P], BF16, kind="ExternalOutput").ap()
        pn_dump = nc.dram_tensor("pn_dump", [65, P], F32, kind="ExternalOutput").ap()
        rb_dump = nc.dram_tensor("rb_dump", [64, P], F32, kind="ExternalOutput").ap()
        zz_dump = nc.dram_tensor("zz_dump", [1, 2 * P], F32, kind="ExternalOutput").ap()

    with tc.tile_pool(name="persist", bufs=1) as persist:
        qT_sb = persist.tile([128, 3, P], BF16)
        outT_sb = [persist.tile([64, P], BF16, name=f"outT{h}", tag=f"outT{h}")
                   for h in range(6)]
        for j in range(3):
            nc.sync.dma_start(out=qT_sb[:, j, :], in_=qT[j * 128:(j + 1) * 128, :])

        with tc.tile_pool(name="kv_store", bufs=1) as kvs:
            kT_sb = kvs.tile([128, 3, N], BF16)
            v_sb = kvs.tile([128, NT, 390], BF16)

            # ---- KV production: one pass over xT ----
            with tc.tile_pool(name="kv_w", bufs=1) as kvw, \
                 tc.tile_pool(name="xstream", bufs=2) as xs, \
                 tc.tile_pool(name="kv_ps", bufs=3, space="PSUM") as kvp, \
                 tc.tile_pool(name="kv_ps_v", bufs=3, space="PSUM") as kvpv:
                wkT_sb = kvw.tile([128, CCH, HG], BF16)
                wvT_sb = kvw.tile([128, CCH, 390], BF16)
                for c in range(CCH):
                    nc.sync.dma_start(out=wkT_sb[:, c, :], in_=wkT[c * 128:(c + 1) * 128, :])
                    nc.sync.dma_start(out=wvT_sb[:, c, :], in_=wvT[c * 128:(c + 1) * 128, :])
                for jc in range(NCH):
                    xt = xs.tile([128, CCH, 512], BF16, tag="xt")
                    for c in range(CCH):
                        nc.sync.dma_start(
                            out=xt[:, c, :],
                            in_=xT[c * 128:(c + 1) * 128, jc * 512:(jc + 1) * 512])
                    for m in range(3):
                        pk = kvp.tile([128, 512], F32, tag="pk")
                        for c in range(CCH):
                            nc.tensor.matmul(
                                pk[:], wkT_sb[:, c, m * 128:(m + 1) * 128], xt[:, c, :],
                                start=(c == 0), stop=(c == CCH - 1))
                        nc.vector.tensor_copy(kT_sb[:, m, jc * 512:(jc + 1) * 512], pk[:])
                    for s in range(4):
                        pv = kvpv.tile([128, 390], F32, tag="pv")
                        for c in range(CCH):
                            nc.tensor.matmul(
                                pv[:], xt[:, c, s * 128:(s + 1) * 128], wvT_sb[:, c, :],
                                start=(c == 0), stop=(c == CCH - 1))
                        it = 4 * jc + s
                        nc.vector.tensor_copy(v_sb[:, it, :], pv[:])
                        # ones columns (64, 129, ..., 389) for the Z row
                        nc.vector.memset(v_sb[:, it, 64:390:65], 1.0)

            if debug:
                nc.sync.dma_start(out=kT_dump[:, :], in_=kT_sb[:, :, :])
                nc.sync.dma_start(out=v_dump[:, :], in_=v_sb[:, :, :])

            # ---- attention, head pair j = heads (2j, 2j+1) ----
            with tc.tile_pool(name="attn", bufs=4) as apool, \
                 tc.tile_pool(name="znorm", bufs=2) as zp, \
                 tc.tile_pool(name="ps_s", bufs=2, space="PSUM") as pss, \
                 tc.tile_pool(name="ps_num", bufs=2, space="PSUM") as psn:
                onec = zp.tile([128, 64], F32, tag="onec")
                nc.vector.memset(onec[64:65, :], 1.0)

                def qk_pair(j, nt):
                    tiles = []
                    for hp in range(2):
                        ps = pss.tile([128, P], F32, name=f"ps{j}_{nt}_{hp}",
                                      tag="ps")
                        lk = kT_sb[hp * 64:(hp + 1) * 64, j,
                                   nt * 128:(nt + 1) * 128]
                        rq = qT_sb[hp * 64:(hp + 1) * 64, j, :]
                        nc.tensor.matmul(ps[:, 0:512], lk, rq[:, 0:512])
                        nc.tensor.matmul(ps[:, 512:1024], lk, rq[:, 512:1024])
                        tiles.append(ps)
                    return tiles

                for j in range(3):
                    pn = [psn.tile([65, P], F32, name=f"pn{j}_{hp}", tag="pn")
                          for hp in range(2)]
                    ps_cur = qk_pair(j, 0)
                    for nt in range(NT):
                        at_list = []
                        for hp in range(2):
                            at = apool.tile([128, P], BF16, tag="at")
                            nc.scalar.activation(at[:], ps_cur[hp][:], EXP)
                            at_list.append(at)
                        if debug and j == 0 and nt == 0:
                            nc.sync.dma_start(out=at_dump[:, :], in_=at_list[0][:])
                        # queue next tile's QK ahead of this tile's AV so the
                        # PE never head-of-line blocks on the exp result
                        ps_next = qk_pair(j, nt + 1) if nt + 1 < NT else None
                        for hp in range(2):
                            va = v_sb[:, nt,
                                      130 * j + 65 * hp: 130 * j + 65 * hp + 65]
                            nc.tensor.matmul(pn[hp][:, 0:512], va,
                                             at_list[hp][:, 0:512],
                                             start=(nt == 0), stop=(nt == NT - 1))
                            nc.tensor.matmul(pn[hp][:, 512:1024], va,
                                             at_list[hp][:, 512:1024],
                                             start=(nt == 0), stop=(nt == NT - 1))
                        ps_cur = ps_next
                    for hp in range(2):
                        h = 2 * j + hp
                        # evacuate PSUM immediately so the next pair's AV can
                        # start; normalization then runs from SBUF off the
                        # critical path
                        pn_sb = zp.tile([65, P], F32, name=f"pnsb{j}_{hp}",
                                        tag="pnsb")
                        nc.vector.tensor_copy(pn_sb[:], pn[hp][:])
                        if debug and j == 0 and hp == 0:
                            nc.sync.dma_start(out=pn_dump[:, :], in_=pn_sb[:])
                        zw = zp.tile([128, 2 * P], F32, tag="zw")
                        nc.vector.tensor_copy(zw[64:65, 0:P], pn_sb[64:65, :])
                        nc.vector.reciprocal(zw[64:65, P:2 * P], zw[64:65, 0:P])
                        # broadcast 1/Z to partitions 0-63 via a K=1 PE matmul
                        prb = pss.tile([64, P], F32, name=f"prb{j}_{hp}", tag="ps")
                        nc.tensor.matmul(prb[:, 0:512], onec[64:65, :],
                                         zw[64:65, P:P + 512])
                        nc.tensor.matmul(prb[:, 512:1024], onec[64:65, :],
                                         zw[64:65, P + 512:2 * P])
                        rb = zp.tile([64, P], F32, tag="rb")
                        nc.vector.tensor_copy(rb[:], prb[:])
                        if debug and j == 0 and hp == 0:
                            nc.sync.dma_start(out=rb_dump[:, :], in_=rb[:])
                            nc.sync.dma_start(out=zz_dump[:, :], in_=zw[64:65, :])
                        nc.vector.tensor_mul(outT_sb[h][:], pn_sb[0:64, :], rb[:])

        # ---- output projection: out[p, :] += sum_h outT_h.T @ projT_h ----
        with tc.tile_pool(name="proj_w", bufs=1) as pj, \
             tc.tile_pool(name="proj_st", bufs=2) as pst, \
             tc.tile_pool(name="proj_ps", bufs=2, space="PSUM") as pjp:
            projT_sb = pj.tile([64, 6, C], BF16)
            for h in range(6):
                nc.sync.dma_start(out=projT_sb[:, h, :], in_=projT[h, :, :])
            for pt in range(P // 128):
                pp = pjp.tile([128, C], F32, tag="pp")
                for h in range(6):
                    lo = outT_sb[h][:, pt * 128:(pt + 1) * 128]
                    nc.tensor.matmul(pp[:, 0:512], lo, projT_sb[:, h, 0:512],
                                     start=(h == 0), stop=(h == 5))
                    nc.tensor.matmul(pp[:, 512:768], lo, projT_sb[:, h, 512:768],
                                     start=(h == 0), stop=(h == 5))
                so = pst.tile([128, C], F32, tag="so")
                nc.vector.tensor_copy(so[:], pp[:])
                nc.sync.dma_start(out=out[pt * 128:(pt + 1) * 128, :], in_=so[:])


def build_program(debug=False, trace_sim=False):
    global _PROGRAM
    if _PROGRAM is not None and not debug:
        return _PROGRAM
    nc = bacc.Bacc("TRN2", target_bir_lowering=False, debug=False, num_devices=8)
    with tile.TileContext(nc, trace_sim=trace_sim) as tc:
        _build_body(tc, debug=debug)
    nc.compile()
    if not debug:
        _PROGRAM = nc
    return nc


def make_in_maps(x, wk_w, wv_w, q_learned, pos_embed, proj_w):
    x = np.ascontiguousarray(np.asarray(x, np.float32))
    wk_w = np.asarray(wk_w, np.float32)
    wv_w = np.asarray(wv_w, np.float32)
    q_learned = np.asarray(q_learned, np.float32)
    pos_embed = np.asarray(pos_embed, np.float32)
    proj_w = np.asarray(proj_w, np.float32)

    q = (q_learned[0, 0][None, :] + pos_embed[0]) * SCALE      # [P, C]
    qT_full = np.ascontiguousarray(q.T)                        # [C, P]
    projT_full = np.ascontiguousarray(proj_w.T)                # [C, C]

    xT_by_b = [np.ascontiguousarray(x[b].T) for b in range(B)]
    in_maps = []
    for core in range(8):
        b, hh = divmod(core, 2)
        sl = slice(hh * HG, (hh + 1) * HG)
        wkT = np.ascontiguousarray(wk_w[sl, :].T)              # [C, 384]
        wvt = wv_w[sl, :].T                                    # [C, 384]
        wvT_aug = np.zeros((C, 390), np.float32)
        for j in range(3):
            wvT_aug[:, 130 * j:130 * j + 64] = wvt[:, 128 * j:128 * j + 64]
            wvT_aug[:, 130 * j + 65:130 * j + 129] = wvt[:, 128 * j + 64:128 * j + 128]
        bf = ml_dtypes.bfloat16
        in_maps.append({
            "xT": xT_by_b[b].astype(bf),
            "wkT": wkT.astype(bf),
            "wvT": wvT_aug.astype(bf),
            "qT": np.ascontiguousarray(qT_full[sl, :]).astype(bf),
            "projT": np.ascontiguousarray(projT_full[sl, :]).reshape(6, 64, C).astype(bf),
        })
    return in_maps


def kernel(x, wk_w, wv_w, q_learned, pos_embed, proj_w, proj_b):
    global LAST_RESULTS
    proj_b = np.asarray(proj_b, np.float32)
    in_maps = make_in_maps(x, wk_w, wv_w, q_learned, pos_embed, proj_w)
    nc = build_program()
    res = run_bass_kernel_spmd(nc, in_maps, core_ids=list(range(8)))
    LAST_RESULTS = res
    parts = [r["out"] for r in res.results]
    out = np.empty((B, P, C), np.float32)
    for b in range(B):
        out[b] = parts[2 * b] + parts[2 * b + 1] + proj_b[None, :]
    return out


# revision 23
# speedup vs baseline: 1.0694x; 1.0694x over previous
"""Trainium2 Bass kernel for nn_CrossAttention_9869834846452.

Cross-attention: learned query (+pos embed) attends over x-derived K/V.
  B=4, N=4096, C=768, H=12 heads, P=1024 queries, head_dim=64.

Sharding over 8 cores: core c = (b, hh) with b = c//2 (batch), hh = c%2
(head-group of 6 heads = 384 channels). Each core computes its 6 heads of
attention for its batch plus the partial output projection over its 384
channels; the host sums the two head-group partials per batch and adds the
projection bias.

Device-side layout choices (all transposes done on host during sharding):
  - x is passed transposed (xT [C, N]) so it serves both as the rhs of the
    kT-production matmul and as the lhsT of the v-production matmul.
  - K is produced directly in transposed form kT [384, N]; scores are
    computed transposed (sT [n, p] = kT_h.T-slices @ qT_h) so that the
    attention*V matmul needs no transposes anywhere.
  - The softmax denominator comes from a ones-column appended to V
    (v_aug [n, 65]); the AV matmul then yields [num; Z] in one pass.
  - Heads are processed in pairs occupying SBUF partitions 0-63 / 64-127,
    which row-packs the K=64 QK matmuls onto disjoint PE row-groups
    (they run concurrently in the array).
"""

import ml_dtypes
import numpy as np

import concourse.bass as bass
import concourse.tile as tile
from concourse import bacc, mybir
from concourse.bass_utils import run_bass_kernel_spmd

F32 = mybir.dt.float32
BF16 = mybir.dt.bfloat16
EXP = mybir.ActivationFunctionType.Exp

B, N, C, H, P, HD = 4, 4096, 768, 12, 1024, 64
SCALE = HD ** -0.5
HG = 384          # channels per head-group (6 heads)
NT = N // 128     # 32 n-tiles
NCH = N // 512    # 8 n-chunks
CCH = C // 128    # 6 contraction chunks

_PROGRAM = None
LAST_RESULTS = None


def _build_body(tc, debug=False):
    nc = tc.nc
    xT = nc.dram_tensor("xT", [C, N], BF16, kind="ExternalInput").ap()
    wkT = nc.dram_tensor("wkT", [C, HG], BF16, kind="ExternalInput").ap()
    wvT = nc.dram_tensor("wvT", [C, 390], BF16, kind="ExternalInput").ap()
    qT = nc.dram_tensor("qT", [HG, P], BF16, kind="ExternalInput").ap()
    projT = nc.dram_tensor("projT", [6, 64, C], BF16, kind="ExternalInput").ap()
    out = nc.dram_tensor("out", [P, C], F32, kind="ExternalOutput").ap()
    if debug:
        kT_dump = nc.dram_tensor("kT_dump", [128, 3 * N], BF16, kind="ExternalOutput").ap()
        v_dump = nc.dram_tensor("v_dump", [128, NT * 390], BF16, kind="ExternalOutput").ap()
        at_dump = nc.dram_tensor("at_dump", [128, P], BF16, kind="ExternalOutput").ap()
        pn_dump = nc.dram_tensor("pn_dump", [65, P], F32, kind="ExternalOutput").ap()
        rb_dump = nc.dram_tensor("rb_dump", [64, P], F32, kind="ExternalOutput").ap()
        zz_dump = nc.dram_tensor("zz_dump", [1, 2 * P], F32, kind="ExternalOutput").ap()

    with tc.tile_pool(name="persist", bufs=1) as persist:
        qT_sb = persist.tile([128, 3, P], BF16)
        outT_sb = [persist.tile([64, P], BF16, name=f"outT{h}", tag=f"outT{h}")
                   for h in range(6)]
        for j in range(3):
            nc.sync.dma_start(out=qT_sb[:, j, :], in_=qT[j * 128:(j + 1) * 128, :])

        with tc.tile_pool(name="kv_store", bufs=1) as kvs:
            kT_sb = kvs.tile([128, 3, N], BF16)
            v_sb = kvs.tile([128, NT, 390], BF16)

            # ---- KV production: one pass over xT ----
            with tc.tile_pool(name="kv_w", bufs=1) as kvw, \
                 tc.tile_pool(name="xstream", bufs=2) as xs, \
                 tc.tile_pool(name="kv_ps", bufs=3, space="PSUM") as kvp, \
                 tc.tile_pool(name="kv_ps_v", bufs=3, space="PSUM") as kvpv:
                wkT_sb = kvw.tile([128, CCH, HG], BF16)
                wvT_sb = kvw.tile([128, CCH, 390], BF16)
                for c in range(CCH):
                    nc.sync.dma_start(out=wkT_sb[:, c, :], in_=wkT[c * 128:(c + 1) * 128, :])
                    nc.sync.dma_start(out=wvT_sb[:, c, :], in_=wvT[c * 128:(c + 1) * 128, :])
                for jc in range(NCH):
                    xt = xs.tile([128, CCH, 512], BF16, tag="xt")
                    for c in range(CCH):
                        nc.sync.dma_start(
                            out=xt[:, c, :],
                            in_=xT[c * 128:(c + 1) * 128, jc * 512:(jc + 1) * 512])
                    for m in range(3):
                        pk = kvp.tile([128, 512], F32, tag="pk")
                        for c in range(CCH):
                            nc.tensor.matmul(
                                pk[:], wkT_sb[:, c, m * 128:(m + 1) * 128], xt[:, c, :],
                                start=(c == 0), stop=(c == CCH - 1))
                        nc.vector.tensor_copy(kT_sb[:, m, jc * 512:(jc + 1) * 512], pk[:])
                    for s in range(4):
                        pv = kvpv.tile([128, 390], F32, tag="pv")
                        for c in range(CCH):
                            nc.tensor.matmul(
                                pv[:], xt[:, c, s * 128:(s + 1) * 128], wvT_sb[:, c, :],
                                start=(c == 0), stop=(c == CCH - 1))
                        it = 4 * jc + s
                        nc.vector.tensor_copy(v_sb[:, it, :], pv[:])
                        # ones columns (64, 129, ..., 389) for the Z row
                        nc.vector.memset(v_sb[:, it, 64:390:65], 1.0)

            if debug:
                nc.sync.dma_start(out=kT_dump[:, :], in_=kT_sb[:, :, :])
                nc.sync.dma_start(out=v_dump[:, :], in_=v_sb[:, :, :])

            # ---- attention, head pair j = heads (2j, 2j+1) ----
            with tc.tile_pool(name="attn", bufs=4) as apool, \
                 tc.tile_pool(name="znorm", bufs=2) as zp, \
                 tc.tile_pool(name="ps_s", bufs=2, space="PSUM") as pss, \
                 tc.tile_pool(name="ps_num", bufs=2, space="PSUM") as psn:
                onec = zp.tile([128, 64], F32, tag="onec")
                nc.vector.memset(onec[64:65, :], 1.0)

                def qk_pair(j, nt):
                    tiles = []
                    for hp in range(2):
                        ps = pss.tile([128, P], F32, name=f"ps{j}_{nt}_{hp}",
                                      tag="ps")
                        lk = kT_sb[hp * 64:(hp + 1) * 64, j,
                                   nt * 128:(nt + 1) * 128]
                        rq = qT_sb[hp * 64:(hp + 1) * 64, j, :]
                        nc.tensor.matmul(ps[:, 0:512], lk, rq[:, 0:512])
                        nc.tensor.matmul(ps[:, 512:1024], lk, rq[:, 512:1024])
                        tiles.append(ps)
                    return tiles

                def norm_tail(j, hp, pn_sb):
                    def run():
                        h = 2 * j + hp
                        zw = zp.tile([128, 2 * P], F32, name=f"zw{j}_{hp}",
                                     tag="zw")
                        nc.vector.tensor_copy(zw[64:65, 0:P], pn_sb[64:65, :])
                        nc.vector.reciprocal(zw[64:65, P:2 * P], zw[64:65, 0:P])
                        # broadcast 1/Z to partitions 0-63 via a K=1 PE matmul
                        prb = pss.tile([64, P], F32, name=f"prb{j}_{hp}",
                                       tag="ps")
                        nc.tensor.matmul(prb[:, 0:512], onec[64:65, :],
                                         zw[64:65, P:P + 512])
                        nc.tensor.matmul(prb[:, 512:1024], onec[64:65, :],
                                         zw[64:65, P + 512:2 * P])
                        rb = zp.tile([64, P], F32, name=f"rb{j}_{hp}", tag="rb")
                        nc.vector.tensor_copy(rb[:], prb[:])
                        if debug and j == 0 and hp == 0:
                            nc.sync.dma_start(out=rb_dump[:, :], in_=rb[:])
                            nc.sync.dma_start(out=zz_dump[:, :], in_=zw[64:65, :])
                        nc.vector.tensor_mul(outT_sb[h][:], pn_sb[0:64, :], rb[:])
                    return run

                pending = []
                for j in range(3):
                    pn = [psn.tile([65, P], F32, name=f"pn{j}_{hp}", tag="pn")
                          for hp in range(2)]
                    ps_cur = qk_pair(j, 0)
                    for nt in range(NT):
                        at_list = []
                        for hp in range(2):
                            at = apool.tile([128, P], BF16, tag="at")
                            nc.scalar.activation(at[:], ps_cur[hp][:], EXP)
                            at_list.append(at)
                        if debug and j == 0 and nt == 0:
                            nc.sync.dma_start(out=at_dump[:, :], in_=at_list[0][:])
                        # queue next tile's QK ahead of this tile's AV so the
                        # PE never head-of-line blocks on the exp result
                        ps_next = qk_pair(j, nt + 1) if nt + 1 < NT else None
                        if nt == 4 and pending:
                            # previous pair's normalization tail, emitted here
                            # so its PE broadcast doesn't stall the boundary
                            for fn in pending:
                                fn()
                            pending = []
                        for hp in range(2):
                            va = v_sb[:, nt,
                                      130 * j + 65 * hp: 130 * j + 65 * hp + 65]
                            nc.tensor.matmul(pn[hp][:, 0:512], va,
                                             at_list[hp][:, 0:512],
                                             start=(nt == 0), stop=(nt == NT - 1))
                            nc.tensor.matmul(pn[hp][:, 512:1024], va,
                                             at_list[hp][:, 512:1024],
                                             start=(nt == 0), stop=(nt == NT - 1))
                        ps_cur = ps_next
                    for hp in range(2):
                        # evacuate PSUM immediately so the next pair's AV can
                        # start; the rest of the normalization is deferred
                        pn_sb = zp.tile([65, P], F32, name=f"pnsb{j}_{hp}",
                                        tag="pnsb")
                        nc.vector.tensor_copy(pn_sb[:], pn[hp][:])
                        if debug and j == 0 and hp == 0:
                            nc.sync.dma_start(out=pn_dump[:, :], in_=pn_sb[:])
                        pending.append(norm_tail(j, hp, pn_sb))
                for fn in pending:
                    fn()

        # ---- output projection: out[p, :] += sum_h outT_h.T @ projT_h ----
        with tc.tile_pool(name="proj_w", bufs=1) as pj, \
             tc.tile_pool(name="proj_st", bufs=2) as pst, \
             tc.tile_pool(name="proj_ps", bufs=2, space="PSUM") as pjp:
            projT_sb = pj.tile([64, 6, C], BF16)
            for h in range(6):
                nc.sync.dma_start(out=projT_sb[:, h, :], in_=projT[h, :, :])
            for pt in range(P // 128):
                pp = pjp.tile([128, C], F32, tag="pp")
                for h in range(6):
                    lo = outT_sb[h][:, pt * 128:(pt + 1) * 128]
                    nc.tensor.matmul(pp[:, 0:512], lo, projT_sb[:, h, 0:512],
                                     start=(h == 0), stop=(h == 5))
                    nc.tensor.matmul(pp[:, 512:768], lo, projT_sb[:, h, 512:768],
                                     start=(h == 0), stop=(h == 5))
                so = pst.tile([128, C], F32, tag="so")
                nc.vector.tensor_copy(so[:], pp[:])
                nc.sync.dma_start(out=out[pt * 128:(pt + 1) * 128, :], in_=so[:])


def build_program(debug=False, trace_sim=False):
    global _PROGRAM
    if _PROGRAM is not None and not debug:
        return _PROGRAM
    nc = bacc.Bacc("TRN2", target_bir_lowering=False, debug=False, num_devices=8)
    with tile.TileContext(nc, trace_sim=trace_sim) as tc:
        _build_body(tc, debug=debug)
    nc.compile()
    if not debug:
        _PROGRAM = nc
    return nc


def make_in_maps(x, wk_w, wv_w, q_learned, pos_embed, proj_w):
    x = np.ascontiguousarray(np.asarray(x, np.float32))
    wk_w = np.asarray(wk_w, np.float32)
    wv_w = np.asarray(wv_w, np.float32)
    q_learned = np.asarray(q_learned, np.float32)
    pos_embed = np.asarray(pos_embed, np.float32)
    proj_w = np.asarray(proj_w, np.float32)

    q = (q_learned[0, 0][None, :] + pos_embed[0]) * SCALE      # [P, C]
    qT_full = np.ascontiguousarray(q.T)                        # [C, P]
    projT_full = np.ascontiguousarray(proj_w.T)                # [C, C]

    xT_by_b = [np.ascontiguousarray(x[b].T) for b in range(B)]
    in_maps = []
    for core in range(8):
        b, hh = divmod(core, 2)
        sl = slice(hh * HG, (hh + 1) * HG)
        wkT = np.ascontiguousarray(wk_w[sl, :].T)              # [C, 384]
        wvt = wv_w[sl, :].T                                    # [C, 384]
        wvT_aug = np.zeros((C, 390), np.float32)
        for j in range(3):
            wvT_aug[:, 130 * j:130 * j + 64] = wvt[:, 128 * j:128 * j + 64]
            wvT_aug[:, 130 * j + 65:130 * j + 129] = wvt[:, 128 * j + 64:128 * j + 128]
        bf = ml_dtypes.bfloat16
        in_maps.append({
            "xT": xT_by_b[b].astype(bf),
            "wkT": wkT.astype(bf),
            "wvT": wvT_aug.astype(bf),
            "qT": np.ascontiguousarray(qT_full[sl, :]).astype(bf),
            "projT": np.ascontiguousarray(projT_full[sl, :]).reshape(6, 64, C).astype(bf),
        })
    return in_maps


def kernel(x, wk_w, wv_w, q_learned, pos_embed, proj_w, proj_b):
    global LAST_RESULTS
    proj_b = np.asarray(proj_b, np.float32)
    in_maps = make_in_maps(x, wk_w, wv_w, q_learned, pos_embed, proj_w)
    nc = build_program()
    res = run_bass_kernel_spmd(nc, in_maps, core_ids=list(range(8)))
    LAST_RESULTS = res
    parts = [r["out"] for r in res.results]
    out = np.empty((B, P, C), np.float32)
    for b in range(B):
        out[b] = parts[2 * b] + parts[2 * b + 1] + proj_b[None, :]
    return out
